# revision 5
# baseline (speedup 1.0000x reference)
"""Trainium2 Bass kernel for dynamic-scale FP8 GEMM (MixLinear):

    out = (scale_in * scale_w) * (q8(x / scale_in) @ q8(w).T) + bias
    scale_in = max|x| / 448  (global over the whole activation tensor)

Strategy (8 NeuronCores, SPMD):
  - Data-parallel over M = B*S = 16384: each core gets a 2048-row shard of x,
    full weight + bias (replicated).
  - On-device global amax: per-core abs-max reduce, then AllReduce(max).
  - TRN fp8_e4m3 saturates at +-240 (vs OCP e4m3fn's +-448), so quantize with
    a 2x scale (values land in +-224) and fold the 2x back at dequant time.
  - x and w are DMA-transposed (fp16, xbar path) into [K-partition, K/128, M|N]
    layout, quantized on-chip to fp8, and the GEMM runs in DoubleRow perf mode
    (contraction 256 per matmul).
  - PSUM is evicted with a single ScalarE activation: out = psum*2s + bias
    (output kept N-major: psum partitions = N-tile), so bias is a per-partition
    scalar.  Per-core output is [N, M_shard]; the host transposes on gather.
"""

import os
import sys

try:
    import concourse  # noqa: F401
except ImportError:  # pragma: no cover
    for _p in ("/opt/trn_rl_repo", "/root/.axon_site/_ro/trn_rl_repo"):
        if os.path.isdir(_p) and _p not in sys.path:
            sys.path.insert(0, _p)

import numpy as np

import concourse.bacc as bacc
import concourse.bass as bass  # noqa: F401
import concourse.mybir as mybir
import concourse.tile as tile
from concourse import bass_isa
from concourse.bass_utils import run_bass_kernel_spmd

# Problem shapes (hardcoded per contract).
B, S, K, N = 4, 4096, 2048, 2048
M = B * S
N_CORES = 8
MS = M // N_CORES  # 2048 rows of x per core

P = 128
F16 = mybir.dt.float16
F32 = mybir.dt.float32
FP8 = mybir.dt.float8e4


def build_nc(ms=MS, k=K, n=N, n_cores=N_CORES):
    """Build + compile the per-core Bass program (SPMD: same NEFF on all cores)."""
    ko = k // P          # k-outer planes
    assert k % 256 == 0 and ms % 1024 == 0 and n % 256 == 0
    m_chunks = ms // 256     # x load/quant chunk count
    n_chunks = n // 256      # w load/quant chunk count
    nt_tiles = n // P        # GEMM stationary n-tiles
    k_pairs = ko // 2        # DoubleRow k steps
    m_half = ms // 2
    mc512 = m_half // 512    # 512-wide m chunks per half

    nc = bacc.Bacc("TRN2", target_bir_lowering=False, debug=False, num_devices=n_cores)
    x = nc.dram_tensor("x", [ms, k], F16, kind="ExternalInput")
    w = nc.dram_tensor("w", [n, k], F16, kind="ExternalInput")
    b = nc.dram_tensor("b", [n], F16, kind="ExternalInput")
    out_t = nc.dram_tensor("out_t", [n, ms], F16, kind="ExternalOutput")

    with tile.TileContext(nc) as tc:
        with (
            tc.tile_pool(name="big", bufs=1) as big,
            tc.tile_pool(name="wtmp", bufs=3) as wtmp,
            tc.tile_pool(name="small", bufs=1) as small,
            tc.tile_pool(name="ev", bufs=4) as ev,
            tc.tile_pool(name="psum", bufs=2, space="PSUM") as psum,
            tc.tile_pool(name="dram", bufs=1, space="DRAM") as dram,
        ):
            # Persistent SBUF tensors.
            xT = big.tile([P, ko, ms], F16)    # x transposed, fp16
            xq = big.tile([P, ko, ms], FP8)    # quantized x (scale 2s)
            wq = big.tile([P, ko, n], FP8)     # quantized w (scale 1)
            acc = small.tile([P, ko * 256], F16)   # running abs-max over x chunks

            # ---- Phase A: transpose-load x, reduce |x| max ----------------
            for mc in range(m_chunks):
                nc.sync.dma_start(
                    out=xT[:, :, mc * 256:(mc + 1) * 256],
                    in_=x.ap()[mc * 256:(mc + 1) * 256, :],
                    transpose=True,
                )
            a3 = acc.rearrange("p (j f) -> p j f", f=256)
            for mc in range(m_chunks):
                ab = wtmp.tile([P, ko, 256], F16, tag="ab", name=f"ab_{mc}")
                nc.scalar.activation(
                    ab[:], xT[:, :, mc * 256:(mc + 1) * 256],
                    mybir.ActivationFunctionType.Abs,
                )
                if mc == 0:
                    nc.vector.tensor_copy(a3, ab[:])
                else:
                    nc.vector.tensor_tensor(a3, a3, ab[:], mybir.AluOpType.max)
            amax_col = small.tile([P, 1], F32)
            nc.vector.tensor_reduce(
                amax_col, acc[:], axis=mybir.AxisListType.X, op=mybir.AluOpType.max
            )
            amax_all = small.tile([P, 1], F32)
            nc.gpsimd.partition_all_reduce(
                amax_all, amax_col, channels=P, reduce_op=bass_isa.ReduceOp.max
            )

            # ---- AllReduce(max) across cores ------------------------------
            cc_in = dram.tile([1], F32)
            cc_addr = "Shared" if n_cores > 4 else "Local"
            cc_out = dram.tile([1], F32, addr_space=cc_addr)
            nc.sync.dma_start(cc_in[:], amax_all[0:1, 0])
            nc.gpsimd.collective_compute(
                "AllReduce",
                mybir.AluOpType.max,
                replica_groups=[list(range(n_cores))],
                ins=[cc_in.opt()],
                outs=[cc_out.opt()],
            )
            scal0 = small.tile([P, 1], F32)
            nc.sync.dma_start(scal0[0:1, :], cc_out[:])
            amax_bc = small.tile([P, 1], F32)
            nc.gpsimd.partition_broadcast(amax_bc, scal0[0:1, :], channels=P)

            # inv2s = 224/amax (quant scale), s2 = amax/224 (dequant scale)
            inv_amax = small.tile([P, 1], F32)
            nc.vector.reciprocal(inv_amax, amax_bc)
            inv2s = small.tile([P, 1], F32)
            nc.vector.tensor_scalar_mul(inv2s, inv_amax, 224.0)
            s2 = small.tile([P, 1], F32)
            nc.vector.tensor_scalar_mul(s2, amax_bc, 1.0 / 224.0)

            # ---- Phase W: transpose-load + quantize weight (scale 1.0) ----
            for nch in range(n_chunks):
                wt = wtmp.tile([P, ko, 256], F16, tag="wt")
                nc.sync.dma_start(
                    out=wt[:],
                    in_=w.ap()[nch * 256:(nch + 1) * 256, :],
                    transpose=True,
                )
                nc.vector.tensor_copy(wq[:, :, nch * 256:(nch + 1) * 256], wt[:])

            # bias -> SBUF [128, n/128] fp32, [p, j] = bias[j*128 + p]
            bias16 = small.tile([P, nt_tiles], F16)
            nc.sync.dma_start(bias16[:], b.ap().rearrange("(j p) -> p j", p=P))
            bias32 = small.tile([P, nt_tiles], F32)
            nc.scalar.copy(bias32[:], bias16[:])

            # ---- Phase Q: quantize x with scale inv2s ---------------------
            for mc in range(m_chunks):
                sl = slice(mc * 256, (mc + 1) * 256)
                nc.scalar.activation(
                    xq[:, :, sl], xT[:, :, sl],
                    mybir.ActivationFunctionType.Copy, scale=inv2s[:],
                )

            # ---- Phase G: fp8 DoubleRow GEMM + fused eviction -------------
            # out_t[nt*128+p, m] = s2 * sum_k wq[k, nt*128+p] * xq[k, m] + bias
            for mh in range(2):
                for nt in range(nt_tiles):
                    ps = [
                        psum.tile([P, 512], F32, tag=f"ps{i}", name=f"ps{i}_{mh}_{nt}")
                        for i in range(mc512)
                    ]
                    for k8 in range(k_pairs):
                        lhsT = wq[:, 2 * k8:2 * k8 + 2, nt * P:(nt + 1) * P]
                        for i2 in range(mc512):
                            m0 = mh * m_half + i2 * 512
                            nc.tensor.matmul(
                                ps[i2][:],
                                lhsT=lhsT,
                                rhs=xq[:, 2 * k8:2 * k8 + 2, m0:m0 + 512],
                                start=(k8 == 0),
                                stop=(k8 == k_pairs - 1),
                                perf_mode=mybir.MatmulPerfMode.DoubleRow,
                            )
                    for i2 in range(mc512):
                        m0 = mh * m_half + i2 * 512
                        ob = ev.tile([P, 512], F16, tag="ob", name=f"ob_{mh}_{nt}_{i2}")
                        nc.scalar.activation(
                            ob[:], ps[i2][:],
                            mybir.ActivationFunctionType.Identity,
                            bias=bias32[:, nt:nt + 1],
                            scale=s2[:],
                        )
                        nc.sync.dma_start(
                            out_t.ap()[nt * P:(nt + 1) * P, m0:m0 + 512], ob[:]
                        )

    nc.compile()
    return nc


_NC_CACHE = {}


def _get_nc():
    if "nc" not in _NC_CACHE:
        _NC_CACHE["nc"] = build_nc()
    return _NC_CACHE["nc"]


def kernel(x, weight, bias):
    x = np.asarray(x, dtype=np.float16).reshape(M, K)
    weight = np.asarray(weight, dtype=np.float16)
    bias = np.asarray(bias, dtype=np.float16)

    nc = _get_nc()
    in_maps = [
        {"x": x[c * MS:(c + 1) * MS], "w": weight, "b": bias}
        for c in range(N_CORES)
    ]
    trace = bool(int(os.environ.get("KERNEL_TRACE", "0")))
    res = run_bass_kernel_spmd(nc, in_maps, list(range(N_CORES)), trace=trace)
    _NC_CACHE["last_result"] = res

    out = np.empty((M, N), dtype=np.float16)
    for c in range(N_CORES):
        out[c * MS:(c + 1) * MS, :] = res.results[c]["out_t"].T
    return out.reshape(B, S, N)


# revision 7
# speedup vs baseline: 1.1207x; 1.1207x over previous
"""Trainium2 Bass kernel for dynamic-scale FP8 GEMM (MixLinear):

    out = (scale_in * scale_w) * (q8(x / scale_in) @ q8(w).T) + bias
    scale_in = max|x| / 448  (global over the whole activation tensor)

Strategy (8 NeuronCores, SPMD):
  - Data-parallel over M = B*S = 16384: each core gets a 2048-row shard of x,
    full weight + bias (replicated).
  - On-device global amax: per-core abs-max reduce, then AllReduce(max).
  - TRN fp8_e4m3 saturates at +-240 (vs OCP e4m3fn's +-448), so quantize with
    a 2x scale (values land in +-224) and fold the 2x back at dequant time.
  - x and w are DMA-transposed (fp16, xbar path) into [K-partition, K/128, M|N]
    layout, quantized on-chip to fp8, and the GEMM runs in DoubleRow perf mode
    (contraction 256 per matmul).
  - PSUM is evicted with a single ScalarE activation: out = psum*2s + bias
    (output kept N-major: psum partitions = N-tile), so bias is a per-partition
    scalar.  Per-core output is [N, M_shard]; the host transposes on gather.
"""

import os
import sys

try:
    import concourse  # noqa: F401
except ImportError:  # pragma: no cover
    for _p in ("/opt/trn_rl_repo", "/root/.axon_site/_ro/trn_rl_repo"):
        if os.path.isdir(_p) and _p not in sys.path:
            sys.path.insert(0, _p)

import numpy as np

import concourse.bacc as bacc
import concourse.bass as bass  # noqa: F401
import concourse.mybir as mybir
import concourse.tile as tile
from concourse import bass_isa
from concourse.bass_utils import run_bass_kernel_spmd

# Problem shapes (hardcoded per contract).
B, S, K, N = 4, 4096, 2048, 2048
M = B * S
N_CORES = 8
MS = M // N_CORES  # 2048 rows of x per core

P = 128
F16 = mybir.dt.float16
F32 = mybir.dt.float32
FP8 = mybir.dt.float8e4


def build_nc(ms=MS, k=K, n=N, n_cores=N_CORES):
    """Build + compile the per-core Bass program (SPMD: same NEFF on all cores)."""
    ko = k // P          # k-outer planes
    assert k % 256 == 0 and ms % 1024 == 0 and n % 256 == 0
    m_chunks = ms // 256     # x load/quant chunk count
    n_chunks = n // 256      # w load/quant chunk count
    nt_tiles = n // P        # GEMM stationary n-tiles
    k_pairs = ko // 2        # DoubleRow k steps
    m_half = ms // 2
    mc512 = m_half // 512    # 512-wide m chunks per half

    nc = bacc.Bacc("TRN2", target_bir_lowering=False, debug=False, num_devices=n_cores)
    x = nc.dram_tensor("x", [ms, k], F16, kind="ExternalInput")
    w = nc.dram_tensor("w", [n, k], F16, kind="ExternalInput")
    b = nc.dram_tensor("b", [n], F16, kind="ExternalInput")
    out_t = nc.dram_tensor("out_t", [n, ms], F16, kind="ExternalOutput")

    with tile.TileContext(nc) as tc:
        with (
            tc.tile_pool(name="big", bufs=1) as big,
            tc.tile_pool(name="wtmp", bufs=3) as wtmp,
            tc.tile_pool(name="small", bufs=1) as small,
            tc.tile_pool(name="ev", bufs=4) as ev,
            tc.tile_pool(name="psum", bufs=2, space="PSUM") as psum,
            tc.tile_pool(name="dram", bufs=1, space="DRAM") as dram,
        ):
            # Persistent SBUF tensors.
            xT = big.tile([P, ko, ms], F16)    # x transposed, fp16
            xq = big.tile([P, ko, ms], FP8)    # quantized x (scale 2s)
            wq = big.tile([P, ko, n], FP8)     # quantized w (scale 1)

            hwdge = [nc.sync, nc.sync]  # transposes on one queue (xbar hazard)

            # ---- Phase A: transpose-load x (both queues), abs-max reduce --
            for mc in range(m_chunks):
                hwdge[mc % 2].dma_start(
                    out=xT[:, :, mc * 256:(mc + 1) * 256],
                    in_=x.ap()[mc * 256:(mc + 1) * 256, :],
                    transpose=True,
                )
            acc_cols = small.tile([P, m_chunks * ko], F32)
            for mc in range(m_chunks):
                nc.vector.tensor_reduce(
                    acc_cols[:, mc * ko:(mc + 1) * ko],
                    xT[:, :, mc * 256:(mc + 1) * 256],
                    axis=mybir.AxisListType.X,
                    op=mybir.AluOpType.max,
                    apply_absolute_value=True,
                )
            amax_col = small.tile([P, 1], F32)
            nc.vector.tensor_reduce(
                amax_col, acc_cols[:], axis=mybir.AxisListType.X,
                op=mybir.AluOpType.max,
            )
            amax_all = small.tile([P, 1], F32)
            nc.gpsimd.partition_all_reduce(
                amax_all, amax_col, channels=P, reduce_op=bass_isa.ReduceOp.max
            )

            # ---- AllGather amaxes across cores, reduce locally ------------
            cc_in = dram.tile([1], F32)
            cc_addr = "Shared" if n_cores > 4 else "Local"
            cc_out = dram.tile([n_cores], F32, addr_space=cc_addr)
            nc.sync.dma_start(cc_in[:], amax_all[0:1, 0])
            nc.gpsimd.collective_compute(
                "AllGather",
                mybir.AluOpType.bypass,
                replica_groups=[list(range(n_cores))],
                ins=[cc_in.opt()],
                outs=[cc_out.opt()],
            )
            scal0 = small.tile([P, n_cores], F32)
            nc.sync.dma_start(scal0[0:1, :], cc_out[:])
            amax1 = small.tile([P, 1], F32)
            nc.vector.tensor_reduce(
                amax1[0:1, :], scal0[0:1, :], axis=mybir.AxisListType.X,
                op=mybir.AluOpType.max,
            )
            amax_bc = small.tile([P, 1], F32)
            nc.gpsimd.partition_broadcast(amax_bc, amax1[0:1, :], channels=P)

            # inv2s = 224/amax (quant scale), s2 = amax/224 (dequant scale)
            inv_amax = small.tile([P, 1], F32)
            nc.vector.reciprocal(inv_amax, amax_bc)
            inv2s = small.tile([P, 1], F32)
            nc.vector.tensor_scalar_mul(inv2s, inv_amax, 224.0)
            s2 = small.tile([P, 1], F32)
            nc.vector.tensor_scalar_mul(s2, amax_bc, 1.0 / 224.0)

            # ---- Phase W: transpose-load + quantize weight (scale 1.0) ----
            for nch in range(n_chunks):
                wt = wtmp.tile([P, ko, 256], F16, tag="wt", bufs=4)
                hwdge[nch % 2].dma_start(
                    out=wt[:],
                    in_=w.ap()[nch * 256:(nch + 1) * 256, :],
                    transpose=True,
                )
                nc.vector.tensor_copy(wq[:, :, nch * 256:(nch + 1) * 256], wt[:])

            # bias -> SBUF [128, n/128] fp32, [p, j] = bias[j*128 + p]
            bias16 = small.tile([P, nt_tiles], F16)
            nc.sync.dma_start(bias16[:], b.ap().rearrange("(j p) -> p j", p=P))
            bias32 = small.tile([P, nt_tiles], F32)
            nc.vector.tensor_copy(bias32[:], bias16[:])

            # ---- Phases Q+G interleaved: quantize a 512-m quarter, GEMM it.
            # Quantization alternates ScalarE (activation w/ scale) and
            # VectorE (tensor_scalar mult) so a quarter is ready in ~4us.
            for mq in range(ms // 512):
                for h in range(2):
                    mc = 2 * mq + h
                    sl = slice(mc * 256, (mc + 1) * 256)
                    if h == 0:
                        nc.scalar.activation(
                            xq[:, :, sl], xT[:, :, sl],
                            mybir.ActivationFunctionType.Copy, scale=inv2s[:],
                        )
                    else:
                        nc.vector.tensor_scalar(
                            xq[:, :, sl], xT[:, :, sl], inv2s[:], None,
                            mybir.AluOpType.mult,
                        )
                m0 = mq * 512
                for nt in range(nt_tiles):
                    ps = psum.tile(
                        [P, 512], F32, tag="ps", bufs=4, name=f"ps_{mq}_{nt}"
                    )
                    for k8 in range(k_pairs):
                        nc.tensor.matmul(
                            ps[:],
                            lhsT=wq[:, 2 * k8:2 * k8 + 2, nt * P:(nt + 1) * P],
                            rhs=xq[:, 2 * k8:2 * k8 + 2, m0:m0 + 512],
                            start=(k8 == 0),
                            stop=(k8 == k_pairs - 1),
                            perf_mode=mybir.MatmulPerfMode.DoubleRow,
                        )
                    ob = ev.tile([P, 512], F16, tag="ob", name=f"ob_{mq}_{nt}")
                    nc.scalar.activation(
                        ob[:], ps[:],
                        mybir.ActivationFunctionType.Identity,
                        bias=bias32[:, nt:nt + 1],
                        scale=s2[:],
                    )
                    nc.sync.dma_start(
                        out_t.ap()[nt * P:(nt + 1) * P, m0:m0 + 512], ob[:]
                    )

    nc.compile()
    return nc


_NC_CACHE = {}


def _get_nc():
    if "nc" not in _NC_CACHE:
        _NC_CACHE["nc"] = build_nc()
    return _NC_CACHE["nc"]


def kernel(x, weight, bias):
    x = np.asarray(x, dtype=np.float16).reshape(M, K)
    weight = np.asarray(weight, dtype=np.float16)
    bias = np.asarray(bias, dtype=np.float16)

    nc = _get_nc()
    in_maps = [
        {"x": x[c * MS:(c + 1) * MS], "w": weight, "b": bias}
        for c in range(N_CORES)
    ]
    trace = bool(int(os.environ.get("KERNEL_TRACE", "0")))
    res = run_bass_kernel_spmd(nc, in_maps, list(range(N_CORES)), trace=trace)
    _NC_CACHE["last_result"] = res

    out = np.empty((M, N), dtype=np.float16)
    for c in range(N_CORES):
        out[c * MS:(c + 1) * MS, :] = res.results[c]["out_t"].T
    return out.reshape(B, S, N)
